# revision 15
# baseline (speedup 1.0000x reference)
"""Multi-head attention (shared QKV projection, floor-div scores) on 8 NeuronCores.

Problem: B=2, S=2048, HID=1024, NH=16, HD=64
    q = k = v = x @ Wq + bq          (reshaped to heads)
    scores = floor(q k^T / sqrt(64)) ; attn = softmax(scores)
    out = (attn v) @ Wo + bo

Sharding: core c handles batch c//4 and 4 heads ((c%4)*4 ..+4). Each core
computes its heads' contribution to out[b] = attn_out @ Wo; the host sums the
4 partials per batch and adds bo.

Device algorithm per core (fp16 matmuls with hi/lo split for full precision):
  - host pre-scales xT by 8^-0.5 (and bq by 8^-0.5, Wo by 8^0.5) so the PE
    score matmuls directly produce s/8 = scores/sqrt(HD); x and Wq are sent
    as fp16 hi + fp16 residual pairs.
  - qT pair tiles: 3-pass fp16 matmul (x16*w16 + dx*w16 + x16*dw) + bias-row
    matmul; PSUM (fp32) split into q16 (fp16) + dq (fp16 residual)
  - v tiles = PE-transposed q16 slices (+ ones column for rowsum Z)
  - per head pair, per q-quarter: scoresT blocks via 3 fp16 matmuls
    (q16*q16 + q16*dq + dq*q16), row-packed across the 2 heads (interleaved
    emission so the two heads' matmuls overlap in the PE array) ->
      floor via RNE trick on DVE: n = (s/8 + 63.5) + 2^23  ->
      P = exp(n - (2^23+64+PSHIFT)) fp16 on ACT (two i-blocks per op) ->
      oT[65, 512] += v_i^T @ P_i  (PE, ones col gives Z row)
    oT evicted to SBUF promptly (frees PSUM); rz = exp(-ln(Z)) (ACT),
    broadcast via DRAM round-trip DMA, oTn = oT * rz fp16 (GPSIMD)
  - partial = oTn_pair^T @ Wo_pair fp16 -> fp16 partial out
"""

import math
import sys

sys.path.insert(0, "/opt/trn_rl_repo")

import numpy as np
import concourse.bass as bass
import concourse.bacc as bacc
import concourse.tile as tile
from concourse import mybir
from concourse.bass_utils import run_bass_kernel_spmd

F32 = mybir.dt.float32
F16 = mybir.dt.float16
ADD = mybir.AluOpType.add
SUB = mybir.AluOpType.subtract
MULT = mybir.AluOpType.mult
AF = mybir.ActivationFunctionType

B, S, HID, NH, HD = 2, 2048, 1024, 16, 64
HPC = 4          # heads per core
NCORES = 8
KT = HID // 128  # 8 k-tiles
QT = S // 128    # 16 q/s tiles
C23 = float(2 ** 23)
PSHIFT = 10.0    # P = e^(n-PSHIFT); cancels in softmax; keeps P < fp16 max
SQ8 = 1.0 / math.sqrt(8.0)

_NC_CACHE = None


def _build():
    nc = bacc.Bacc("TRN2", target_bir_lowering=False, debug=False,
                   num_devices=NCORES)

    x16d = nc.dram_tensor("x16", [HID, S], F16, kind="ExternalInput")
    dx16d = nc.dram_tensor("dx16", [HID, S], F16, kind="ExternalInput")
    wqd, dwqd, wod, bqrd = [], [], [], []
    for p in range(2):
        wqd.append(nc.dram_tensor(f"wq{p}", [128, 1024], F16,
                                  kind="ExternalInput"))
        dwqd.append(nc.dram_tensor(f"dwq{p}", [128, 1024], F16,
                                   kind="ExternalInput"))
        wod.append(nc.dram_tensor(f"wo{p}", [128, 1024], F16,
                                  kind="ExternalInput"))
        bqrd.append(nc.dram_tensor(f"bqr{p}", [1, 128], F16,
                                   kind="ExternalInput"))
    ident = nc.dram_tensor("ident", [128, 64], F16, kind="ExternalInput")
    part = nc.dram_tensor("part", [S, HID], F16, kind="ExternalOutput")
    rzscr = nc.dram_tensor("rzscr", [HPC, S], F32)

    with tile.TileContext(nc) as tc:
        with (
            tc.tile_pool(name="cst", bufs=1) as cst,
            tc.tile_pool(name="big", bufs=1) as big,
            tc.tile_pool(name="wrk", bufs=3) as wrk,
            tc.tile_pool(name="osb", bufs=2) as osb,
            tc.tile_pool(name="zs", bufs=1) as zs,
            tc.tile_pool(name="ps_big", bufs=2, space="PSUM") as ps_big,
            tc.tile_pool(name="ps_small", bufs=3, space="PSUM") as ps_small,
        ):
            # ---- constants / inputs ----
            b_negc = cst.tile([128, 1], F32, tag="b_negc")
            nc.vector.memset(b_negc[:], -(C23 + 64.0 + PSHIFT))
            onesr = cst.tile([1, S], F16, tag="onesr")
            nc.vector.memset(onesr[:], 1.0)
            id_t = cst.tile([128, 64], F16, tag="id_t")
            nc.sync.dma_start(id_t[:], ident[:])
            wq_t, dwq_t, wo_t, bqr_t = [], [], [], []
            for p in range(2):
                w = cst.tile([128, 1024], F16, tag=f"wq_t{p}")
                nc.sync.dma_start(w[:], wqd[p][:])
                wq_t.append(w)
                w = cst.tile([128, 1024], F16, tag=f"dwq_t{p}")
                nc.sync.dma_start(w[:], dwqd[p][:])
                dwq_t.append(w)
                w = cst.tile([128, 1024], F16, tag=f"wo_t{p}")
                nc.sync.dma_start(w[:], wod[p][:])
                wo_t.append(w)
                w = cst.tile([1, 128], F16, tag=f"bqr_t{p}")
                nc.sync.dma_start(w[:], bqrd[p][:])
                bqr_t.append(w)
            x16_t, dx16_t = [], []
            for t in range(KT):
                w = big.tile([128, S], F16, tag=f"x16_{t}")
                nc.sync.dma_start(w[:], x16d[t * 128:(t + 1) * 128, :])
                x16_t.append(w)
            for t in range(KT):
                w = big.tile([128, S], F16, tag=f"dx16_{t}")
                nc.sync.dma_start(w[:], dx16d[t * 128:(t + 1) * 128, :])
                dx16_t.append(w)

            # ---- phase 1: qT projection (3-pass fp16) -> q16 + dq ----
            q16_t, dq_t = [], []
            for p in range(2):
                w = big.tile([128, S], F16, tag=f"q16_{p}")
                q16_t.append(w)
                w = big.tile([128, S], F16, tag=f"dq_{p}")
                dq_t.append(w)
            for p in range(2):
                for jh in range(2):
                    o = jh * 1024
                    pq = ps_small.tile([128, 1024], F32, tag="smallps",
                                       name=f"pq{p}_{jh}")
                    for t in range(KT):
                        for rx in (x16_t, dx16_t):
                            for c in range(2):
                                nc.tensor.matmul(
                                    pq[:, c * 512:(c + 1) * 512],
                                    wq_t[p][:, t * 128:(t + 1) * 128],
                                    rx[t][:, o + c * 512:o + (c + 1) * 512],
                                    start=(t == 0 and rx is x16_t),
                                    stop=False,
                                )
                    for t in range(KT):
                        for c in range(2):
                            nc.tensor.matmul(
                                pq[:, c * 512:(c + 1) * 512],
                                dwq_t[p][:, t * 128:(t + 1) * 128],
                                x16_t[t][:, o + c * 512:o + (c + 1) * 512],
                                start=False, stop=False,
                            )
                    for c in range(2):
                        nc.tensor.matmul(
                            pq[:, c * 512:(c + 1) * 512],
                            bqr_t[p][:],
                            onesr[:, o + c * 512:o + (c + 1) * 512],
                            start=False, stop=(c == 1),
                        )
                    nc.scalar.activation(q16_t[p][:, o:o + 1024], pq[:],
                                         AF.Identity, bias=0.0, scale=1.0)
                    nc.vector.tensor_tensor(dq_t[p][:, o:o + 1024], pq[:],
                                            q16_t[p][:, o:o + 1024], SUB)

            # ---- phase 2: v tiles (transposed q16 + ones col) ----
            v_t = []
            for h in range(HPC):
                p, r = h // 2, (h % 2) * 64
                vt = big.tile([128, QT * 65], F16, tag=f"v{h}")
                for half in range(2):
                    pv = ps_small.tile([128, 512], F16, tag="smallps",
                                       name=f"pv{h}_{half}")
                    for tt in range(8):
                        i = half * 8 + tt
                        nc.tensor.transpose(
                            pv[:, tt * 64:(tt + 1) * 64],
                            q16_t[p][r:r + 64, i * 128:(i + 1) * 128],
                            id_t[r:r + 64, :],
                        )
                    dst = (vt[:, half * 520:half * 520 + 520]
                           .rearrange("p (t e) -> p t e", e=65)[:, :, 0:64])
                    src = pv[:, 0:512].rearrange("p (t e) -> p t e", e=64)
                    nc.vector.tensor_copy(dst, src)
                ones = vt[:].rearrange("p (t e) -> p t e", e=65)[:, :, 64:65]
                nc.vector.memset(ones, 1.0)
                v_t.append(vt)

            # ---- phase 3: per head-pair, per q-quarter ----
            oTn_t = []
            for p in range(2):
                w = big.tile([128, S], F16, tag=f"oTn{p}")
                oTn_t.append(w)

            for pp in range(2):
                q16 = q16_t[pp]
                dq = dq_t[pp]
                oTsb = []
                for hx in range(2):
                    w = osb.tile([65, S], F32, tag="oTsb",
                                 name=f"oTsb{pp}_{hx}")
                    oTsb.append(w)
                for J in range(4):
                    jo = J * 512
                    poT = []
                    for hx in range(2):
                        w = ps_big.tile([65, 512], F32, tag="bigps",
                                        name=f"poT{pp}_{J}_{hx}")
                        poT.append(w)
                    nt = None
                    for i in range(QT):
                        half = i % 2
                        if half == 0:
                            nt = wrk.tile([128, 2048], F32, tag="ntile")
                        pss = ps_small.tile([128, 1024], F32, tag="smallps",
                                            name=f"pss{pp}_{J}_{i}")
                        itile = slice(i * 128, (i + 1) * 128)
                        cs = slice(jo, jo + 512)
                        # 3-pass scores, interleaved across the head pair so
                        # the two row-groups overlap in the PE array
                        for pa, (wsel, rsel) in enumerate(
                                [(q16, q16), (q16, dq), (dq, q16)]):
                            for hx in range(2):
                                rs = slice(hx * 64, hx * 64 + 64)
                                nc.tensor.matmul(
                                    pss[:, hx * 512:(hx + 1) * 512],
                                    wsel[rs, itile], rsel[rs, cs],
                                    start=(pa == 0), stop=(pa == 2),
                                )
                        nc.vector.tensor_scalar(
                            nt[:, half * 1024:(half + 1) * 1024], pss[:],
                            63.5, C23, ADD, ADD,
                        )
                        if half == 1:
                            pt = wrk.tile([128, 2048], F16, tag="ptile")
                            nc.scalar.activation(pt[:], nt[:], AF.Exp,
                                                 bias=b_negc[:], scale=1.0)
                            for ii, hx in ((i - 1, 0), (i - 1, 1),
                                           (i, 0), (i, 1)):
                                h = 2 * pp + hx
                                hf = (ii % 2) * 1024
                                nc.tensor.matmul(
                                    poT[hx][:],
                                    v_t[h][:, ii * 65:(ii + 1) * 65],
                                    pt[:, hf + hx * 512:hf + (hx + 1) * 512],
                                    start=(ii == 0), stop=(ii == QT - 1),
                                )
                    for hx in range(2):
                        if (J + hx) % 2 == 0:
                            nc.scalar.copy(oTsb[hx][:, jo:jo + 512], poT[hx][:])
                        else:
                            nc.vector.tensor_copy(oTsb[hx][:, jo:jo + 512],
                                                  poT[hx][:])

                # normalization per head of this pair
                for hx in range(2):
                    h = 2 * pp + hx
                    r = hx * 64
                    lnz = zs.tile([1, S], F32, tag="lnz")
                    nc.scalar.activation(lnz[:], oTsb[hx][64:65, :], AF.Ln,
                                         bias=0.0, scale=1.0)
                    rz = zs.tile([1, S], F32, tag="rz")
                    nc.scalar.activation(rz[:], lnz[:], AF.Exp,
                                         bias=0.0, scale=-1.0)
                    if pp == 0:
                        # off the critical path: DMA round-trip broadcast
                        nc.sync.dma_start(rzscr[h:h + 1, :], rz[:])
                        repz = zs.tile([64, S], F32, tag="repz")
                        nc.sync.dma_start(
                            repz[:], rzscr[h:h + 1, :].broadcast_to([64, S]))
                        nc.gpsimd.tensor_tensor(oTn_t[pp][r:r + 64, :],
                                                oTsb[hx][0:64, :], repz[:],
                                                MULT)
                    else:
                        # tail-critical: PE rank-1 broadcast + DVE multiply
                        rz16 = zs.tile([1, S], F16, tag="rz16")
                        nc.vector.tensor_copy(rz16[:], rz[:])
                        for jh in range(2):
                            o2 = jh * 1024
                            prz = ps_small.tile([64, 1024], F32, tag="smallps",
                                                name=f"prz{hx}_{jh}")
                            for c in range(2):
                                nc.tensor.matmul(
                                    prz[:, c * 512:(c + 1) * 512],
                                    onesr[0:1, 0:64],
                                    rz16[:, o2 + c * 512:o2 + (c + 1) * 512],
                                    start=True, stop=True,
                                )
                            nc.vector.tensor_tensor(
                                oTn_t[pp][r:r + 64, o2:o2 + 1024],
                                prz[:], oTsb[hx][0:64, o2:o2 + 1024], MULT)

            # ---- phase 4: output projection (fp16) ----
            for m in range(QT):
                po = ps_small.tile([128, 1024], F32, tag="smallps",
                                   name=f"po{m}")
                for c in range(2):
                    nc.tensor.matmul(
                        po[:, c * 512:(c + 1) * 512],
                        oTn_t[0][:, m * 128:(m + 1) * 128],
                        wo_t[0][:, c * 512:(c + 1) * 512],
                        start=True, stop=False,
                    )
                    nc.tensor.matmul(
                        po[:, c * 512:(c + 1) * 512],
                        oTn_t[1][:, m * 128:(m + 1) * 128],
                        wo_t[1][:, c * 512:(c + 1) * 512],
                        start=False, stop=True,
                    )
                ot = wrk.tile([128, 1024], F16, tag="ptile", name=f"ostage{m}")
                if m % 2 == 0:
                    nc.vector.tensor_copy(ot[:], po[:])
                else:
                    nc.scalar.copy(ot[:], po[:])
                nc.sync.dma_start(part[m * 128:(m + 1) * 128, :], ot[:])

    nc.finalize()
    return nc


def _get_nc():
    global _NC_CACHE
    if _NC_CACHE is None:
        _NC_CACHE = _build()
    return _NC_CACHE


def make_in_maps(x, Wq, bq, Wo):
    eye = np.eye(64, dtype=np.float16)
    ident = np.vstack([eye, eye])
    in_maps = []
    for c in range(NCORES):
        b, hb = c // 4, (c % 4) * HPC
        xts = np.ascontiguousarray(x[b].T) * np.float32(SQ8)   # [1024, 2048]
        x16 = xts.astype(np.float16)
        dx16 = (xts - x16.astype(np.float32)).astype(np.float16)
        m = {"x16": x16, "dx16": dx16, "ident": ident}
        for p in range(2):
            lo = (hb + 2 * p) * HD          # first col/row of this head pair
            wq_cols = Wq[:, lo:lo + 128]    # [1024, 128]
            # lhsT k-tile layout: [128 part, 8 ktiles x 128]
            wqp = np.ascontiguousarray(
                wq_cols.reshape(KT, 128, 128).transpose(1, 0, 2).reshape(128, 1024)
            )
            w16 = wqp.astype(np.float16)
            m[f"wq{p}"] = w16
            m[f"dwq{p}"] = (wqp - w16.astype(np.float32)).astype(np.float16)
            m[f"wo{p}"] = (np.ascontiguousarray(Wo[lo:lo + 128, :])
                           * np.float32(1.0 / SQ8)).astype(np.float16)
            m[f"bqr{p}"] = (bq[None, lo:lo + 128]
                            * np.float32(SQ8)).astype(np.float16)
        in_maps.append(m)
    return in_maps


def kernel(x, Wq, bq, Wo, bo):
    x = np.asarray(x, np.float32)
    Wq = np.asarray(Wq, np.float32)
    bq = np.asarray(bq, np.float32)
    Wo = np.asarray(Wo, np.float32)
    bo = np.asarray(bo, np.float32)

    in_maps = make_in_maps(x, Wq, bq, Wo)
    res = run_bass_kernel_spmd(_get_nc(), in_maps, list(range(NCORES)))
    parts = [r["part"] for r in res.results]
    out = np.empty((B, S, HID), np.float32)
    for b in range(B):
        out[b] = (parts[4 * b].astype(np.float32)
                  + parts[4 * b + 1].astype(np.float32)
                  + parts[4 * b + 2].astype(np.float32)
                  + parts[4 * b + 3].astype(np.float32))
        out[b] += bo[None, :]
    return out


# revision 16
# speedup vs baseline: 1.0244x; 1.0244x over previous
"""Multi-head attention (shared QKV projection, floor-div scores) on 8 NeuronCores.

Problem: B=2, S=2048, HID=1024, NH=16, HD=64
    q = k = v = x @ Wq + bq          (reshaped to heads)
    scores = floor(q k^T / sqrt(64)) ; attn = softmax(scores)
    out = (attn v) @ Wo + bo

Sharding: core c handles batch c//4 and 4 heads ((c%4)*4 ..+4). Each core
computes its heads' contribution to out[b] = attn_out @ Wo; the host sums the
4 partials per batch and adds bo.

Device algorithm per core (fp16 matmuls with hi/lo split for full precision):
  - host pre-scales xT by 8^-0.5 (and bq by 8^-0.5, Wo by 8^0.5) so the PE
    score matmuls directly produce s/8 = scores/sqrt(HD); x and Wq are sent
    as fp16 hi + fp16 residual pairs.
  - qT pair tiles: 3-pass fp16 matmul (x16*w16 + dx*w16 + x16*dw) + bias-row
    matmul; PSUM (fp32) split into q16 (fp16) + dq (fp16 residual)
  - v tiles = PE-transposed q16 slices (+ ones column for rowsum Z)
  - per head pair, per q-quarter: scoresT blocks via 3 fp16 matmuls
    (q16*q16 + q16*dq + dq*q16), row-packed across the 2 heads (interleaved
    emission so the two heads' matmuls overlap in the PE array) ->
      floor via RNE trick on DVE: n = (s/8 + 63.5) + 2^23  ->
      P = exp(n - (2^23+64+PSHIFT)) fp16 on ACT (two i-blocks per op) ->
      oT[65, 512] += v_i^T @ P_i  (PE, ones col gives Z row)
    oT evicted to SBUF promptly (frees PSUM); rz = exp(-ln(Z)) (ACT),
    broadcast via DRAM round-trip DMA, oTn = oT * rz fp16 (GPSIMD)
  - partial = oTn_pair^T @ Wo_pair fp16 -> fp16 partial out
"""

import math
import sys

sys.path.insert(0, "/opt/trn_rl_repo")

import numpy as np
import concourse.bass as bass
import concourse.bacc as bacc
import concourse.tile as tile
from concourse import mybir
from concourse.bass_utils import run_bass_kernel_spmd

F32 = mybir.dt.float32
F16 = mybir.dt.float16
ADD = mybir.AluOpType.add
SUB = mybir.AluOpType.subtract
MULT = mybir.AluOpType.mult
AF = mybir.ActivationFunctionType

B, S, HID, NH, HD = 2, 2048, 1024, 16, 64
HPC = 4          # heads per core
NCORES = 8
KT = HID // 128  # 8 k-tiles
QT = S // 128    # 16 q/s tiles
C23 = float(2 ** 23)
PSHIFT = 10.0    # P = e^(n-PSHIFT); cancels in softmax; keeps P < fp16 max
SQ8 = 1.0 / math.sqrt(8.0)

_NC_CACHE = None


def _build():
    nc = bacc.Bacc("TRN2", target_bir_lowering=False, debug=False,
                   num_devices=NCORES)

    x16d = nc.dram_tensor("x16", [HID, S], F16, kind="ExternalInput")
    dx16d = nc.dram_tensor("dx16", [HID, S], F16, kind="ExternalInput")
    wqd, dwqd, wod, bqrd = [], [], [], []
    for p in range(2):
        wqd.append(nc.dram_tensor(f"wq{p}", [128, 1024], F16,
                                  kind="ExternalInput"))
        dwqd.append(nc.dram_tensor(f"dwq{p}", [128, 1024], F16,
                                   kind="ExternalInput"))
        wod.append(nc.dram_tensor(f"wo{p}", [128, 1024], F16,
                                  kind="ExternalInput"))
        bqrd.append(nc.dram_tensor(f"bqr{p}", [1, 128], F16,
                                   kind="ExternalInput"))
    ident = nc.dram_tensor("ident", [128, 64], F16, kind="ExternalInput")
    part = nc.dram_tensor("part", [S, HID], F16, kind="ExternalOutput")
    rzscr = nc.dram_tensor("rzscr", [HPC, S], F32)

    with tile.TileContext(nc) as tc:
        with (
            tc.tile_pool(name="cst", bufs=1) as cst,
            tc.tile_pool(name="big", bufs=1) as big,
            tc.tile_pool(name="wrk", bufs=3) as wrk,
            tc.tile_pool(name="osb", bufs=2) as osb,
            tc.tile_pool(name="zs", bufs=1) as zs,
            tc.tile_pool(name="ps_big", bufs=2, space="PSUM") as ps_big,
            tc.tile_pool(name="ps_small", bufs=3, space="PSUM") as ps_small,
        ):
            # ---- constants / inputs ----
            b_negc = cst.tile([128, 1], F32, tag="b_negc")
            nc.vector.memset(b_negc[:], -(C23 + 64.0 + PSHIFT))
            onesr = cst.tile([1, S], F16, tag="onesr")
            nc.vector.memset(onesr[:], 1.0)
            id_t = cst.tile([128, 64], F16, tag="id_t")
            nc.sync.dma_start(id_t[:], ident[:])
            wq_t, dwq_t, wo_t, bqr_t = [], [], [], []
            for p in range(2):
                w = cst.tile([128, 1024], F16, tag=f"wq_t{p}")
                nc.sync.dma_start(w[:], wqd[p][:])
                wq_t.append(w)
                w = cst.tile([128, 1024], F16, tag=f"dwq_t{p}")
                nc.sync.dma_start(w[:], dwqd[p][:])
                dwq_t.append(w)
                w = cst.tile([128, 1024], F16, tag=f"wo_t{p}")
                nc.sync.dma_start(w[:], wod[p][:])
                wo_t.append(w)
                w = cst.tile([1, 128], F16, tag=f"bqr_t{p}")
                nc.sync.dma_start(w[:], bqrd[p][:])
                bqr_t.append(w)
            x16_t, dx16_t = [], []
            for t in range(KT):
                w = big.tile([128, S], F16, tag=f"x16_{t}")
                nc.sync.dma_start(w[:], x16d[t * 128:(t + 1) * 128, :])
                x16_t.append(w)
            for t in range(KT):
                w = big.tile([128, S], F16, tag=f"dx16_{t}")
                nc.sync.dma_start(w[:], dx16d[t * 128:(t + 1) * 128, :])
                dx16_t.append(w)

            # ---- phase 1: qT projection (3-pass fp16) -> q16 + dq ----
            q16_t, dq_t = [], []
            for p in range(2):
                w = big.tile([128, S], F16, tag=f"q16_{p}")
                q16_t.append(w)
                w = big.tile([128, S], F16, tag=f"dq_{p}")
                dq_t.append(w)
            for p in range(2):
                for jh in range(2):
                    o = jh * 1024
                    pq = ps_small.tile([128, 1024], F32, tag="smallps",
                                       name=f"pq{p}_{jh}")
                    for pa, (lw, rx) in enumerate(
                            [(wq_t[p], x16_t), (dwq_t[p], x16_t),
                             (wq_t[p], dx16_t)]):
                        for t in range(KT):
                            for c in range(2):
                                nc.tensor.matmul(
                                    pq[:, c * 512:(c + 1) * 512],
                                    lw[:, t * 128:(t + 1) * 128],
                                    rx[t][:, o + c * 512:o + (c + 1) * 512],
                                    start=(pa == 0 and t == 0), stop=False,
                                )
                    for c in range(2):
                        nc.tensor.matmul(
                            pq[:, c * 512:(c + 1) * 512],
                            bqr_t[p][:],
                            onesr[:, o + c * 512:o + (c + 1) * 512],
                            start=False, stop=(c == 1),
                        )
                    nc.scalar.activation(q16_t[p][:, o:o + 1024], pq[:],
                                         AF.Identity, bias=0.0, scale=1.0)
                    nc.vector.tensor_tensor(dq_t[p][:, o:o + 1024], pq[:],
                                            q16_t[p][:, o:o + 1024], SUB)

            # ---- phase 2: v tiles (transposed q16 + ones col) ----
            v_t = []
            for h in range(HPC):
                p, r = h // 2, (h % 2) * 64
                vt = big.tile([128, QT * 65], F16, tag=f"v{h}")
                for half in range(2):
                    pv = ps_small.tile([128, 512], F16, tag="smallps",
                                       name=f"pv{h}_{half}")
                    for tt in range(8):
                        i = half * 8 + tt
                        nc.tensor.transpose(
                            pv[:, tt * 64:(tt + 1) * 64],
                            q16_t[p][r:r + 64, i * 128:(i + 1) * 128],
                            id_t[r:r + 64, :],
                        )
                    dst = (vt[:, half * 520:half * 520 + 520]
                           .rearrange("p (t e) -> p t e", e=65)[:, :, 0:64])
                    src = pv[:, 0:512].rearrange("p (t e) -> p t e", e=64)
                    nc.vector.tensor_copy(dst, src)
                ones = vt[:].rearrange("p (t e) -> p t e", e=65)[:, :, 64:65]
                nc.vector.memset(ones, 1.0)
                v_t.append(vt)

            # ---- phase 3: per head-pair, per q-quarter ----
            oTn_t = []
            for p in range(2):
                w = big.tile([128, S], F16, tag=f"oTn{p}")
                oTn_t.append(w)

            for pp in range(2):
                q16 = q16_t[pp]
                dq = dq_t[pp]
                oTsb = []
                for hx in range(2):
                    w = osb.tile([65, S], F32, tag="oTsb",
                                 name=f"oTsb{pp}_{hx}")
                    oTsb.append(w)
                for J in range(4):
                    jo = J * 512
                    poT = []
                    for hx in range(2):
                        w = ps_big.tile([65, 512], F32, tag="bigps",
                                        name=f"poT{pp}_{J}_{hx}")
                        poT.append(w)
                    nt = None
                    for i in range(QT):
                        half = i % 2
                        if half == 0:
                            nt = wrk.tile([128, 2048], F32, tag="ntile")
                        pss = ps_small.tile([128, 1024], F32, tag="smallps",
                                            name=f"pss{pp}_{J}_{i}")
                        itile = slice(i * 128, (i + 1) * 128)
                        cs = slice(jo, jo + 512)
                        # 3-pass scores, interleaved across the head pair so
                        # the two row-groups overlap in the PE array
                        for pa, (wsel, rsel) in enumerate(
                                [(q16, q16), (q16, dq), (dq, q16)]):
                            for hx in range(2):
                                rs = slice(hx * 64, hx * 64 + 64)
                                nc.tensor.matmul(
                                    pss[:, hx * 512:(hx + 1) * 512],
                                    wsel[rs, itile], rsel[rs, cs],
                                    start=(pa == 0), stop=(pa == 2),
                                )
                        nc.vector.tensor_scalar(
                            nt[:, half * 1024:(half + 1) * 1024], pss[:],
                            63.5, C23, ADD, ADD,
                        )
                        if half == 1:
                            pt = wrk.tile([128, 2048], F16, tag="ptile")
                            nc.scalar.activation(pt[:], nt[:], AF.Exp,
                                                 bias=b_negc[:], scale=1.0)
                            for ii, hx in ((i - 1, 0), (i - 1, 1),
                                           (i, 0), (i, 1)):
                                h = 2 * pp + hx
                                hf = (ii % 2) * 1024
                                nc.tensor.matmul(
                                    poT[hx][:],
                                    v_t[h][:, ii * 65:(ii + 1) * 65],
                                    pt[:, hf + hx * 512:hf + (hx + 1) * 512],
                                    start=(ii == 0), stop=(ii == QT - 1),
                                )
                    for hx in range(2):
                        if (J + hx) % 2 == 0:
                            nc.scalar.copy(oTsb[hx][:, jo:jo + 512], poT[hx][:])
                        else:
                            nc.vector.tensor_copy(oTsb[hx][:, jo:jo + 512],
                                                  poT[hx][:])

                # normalization per head of this pair
                for hx in range(2):
                    h = 2 * pp + hx
                    r = hx * 64
                    lnz = zs.tile([1, S], F32, tag="lnz")
                    nc.scalar.activation(lnz[:], oTsb[hx][64:65, :], AF.Ln,
                                         bias=0.0, scale=1.0)
                    rz = zs.tile([1, S], F32, tag="rz")
                    nc.scalar.activation(rz[:], lnz[:], AF.Exp,
                                         bias=0.0, scale=-1.0)
                    if pp == 0:
                        # off the critical path: DMA round-trip broadcast
                        nc.sync.dma_start(rzscr[h:h + 1, :], rz[:])
                        repz = zs.tile([64, S], F32, tag="repz")
                        nc.sync.dma_start(
                            repz[:], rzscr[h:h + 1, :].broadcast_to([64, S]))
                        nc.gpsimd.tensor_tensor(oTn_t[pp][r:r + 64, :],
                                                oTsb[hx][0:64, :], repz[:],
                                                MULT)
                    else:
                        # tail-critical: PE rank-1 broadcast + DVE multiply
                        rz16 = zs.tile([1, S], F16, tag="rz16")
                        nc.vector.tensor_copy(rz16[:], rz[:])
                        for jh in range(2):
                            o2 = jh * 1024
                            prz = ps_small.tile([64, 1024], F32, tag="smallps",
                                                name=f"prz{hx}_{jh}")
                            for c in range(2):
                                nc.tensor.matmul(
                                    prz[:, c * 512:(c + 1) * 512],
                                    onesr[0:1, 0:64],
                                    rz16[:, o2 + c * 512:o2 + (c + 1) * 512],
                                    start=True, stop=True,
                                )
                            nc.vector.tensor_tensor(
                                oTn_t[pp][r:r + 64, o2:o2 + 1024],
                                prz[:], oTsb[hx][0:64, o2:o2 + 1024], MULT)

            # ---- phase 4: output projection (fp16) ----
            for m in range(QT):
                po = ps_small.tile([128, 1024], F32, tag="smallps",
                                   name=f"po{m}")
                for c in range(2):
                    nc.tensor.matmul(
                        po[:, c * 512:(c + 1) * 512],
                        oTn_t[0][:, m * 128:(m + 1) * 128],
                        wo_t[0][:, c * 512:(c + 1) * 512],
                        start=True, stop=False,
                    )
                    nc.tensor.matmul(
                        po[:, c * 512:(c + 1) * 512],
                        oTn_t[1][:, m * 128:(m + 1) * 128],
                        wo_t[1][:, c * 512:(c + 1) * 512],
                        start=False, stop=True,
                    )
                ot = wrk.tile([128, 1024], F16, tag="ptile", name=f"ostage{m}")
                if m % 2 == 0:
                    nc.vector.tensor_copy(ot[:], po[:])
                else:
                    nc.scalar.copy(ot[:], po[:])
                nc.sync.dma_start(part[m * 128:(m + 1) * 128, :], ot[:])

    nc.finalize()
    return nc


def _get_nc():
    global _NC_CACHE
    if _NC_CACHE is None:
        _NC_CACHE = _build()
    return _NC_CACHE


def make_in_maps(x, Wq, bq, Wo):
    eye = np.eye(64, dtype=np.float16)
    ident = np.vstack([eye, eye])
    in_maps = []
    for c in range(NCORES):
        b, hb = c // 4, (c % 4) * HPC
        xts = np.ascontiguousarray(x[b].T) * np.float32(SQ8)   # [1024, 2048]
        x16 = xts.astype(np.float16)
        dx16 = (xts - x16.astype(np.float32)).astype(np.float16)
        m = {"x16": x16, "dx16": dx16, "ident": ident}
        for p in range(2):
            lo = (hb + 2 * p) * HD          # first col/row of this head pair
            wq_cols = Wq[:, lo:lo + 128]    # [1024, 128]
            # lhsT k-tile layout: [128 part, 8 ktiles x 128]
            wqp = np.ascontiguousarray(
                wq_cols.reshape(KT, 128, 128).transpose(1, 0, 2).reshape(128, 1024)
            )
            w16 = wqp.astype(np.float16)
            m[f"wq{p}"] = w16
            m[f"dwq{p}"] = (wqp - w16.astype(np.float32)).astype(np.float16)
            m[f"wo{p}"] = (np.ascontiguousarray(Wo[lo:lo + 128, :])
                           * np.float32(1.0 / SQ8)).astype(np.float16)
            m[f"bqr{p}"] = (bq[None, lo:lo + 128]
                            * np.float32(SQ8)).astype(np.float16)
        in_maps.append(m)
    return in_maps


def kernel(x, Wq, bq, Wo, bo):
    x = np.asarray(x, np.float32)
    Wq = np.asarray(Wq, np.float32)
    bq = np.asarray(bq, np.float32)
    Wo = np.asarray(Wo, np.float32)
    bo = np.asarray(bo, np.float32)

    in_maps = make_in_maps(x, Wq, bq, Wo)
    res = run_bass_kernel_spmd(_get_nc(), in_maps, list(range(NCORES)))
    parts = [r["part"] for r in res.results]
    out = np.empty((B, S, HID), np.float32)
    for b in range(B):
        out[b] = (parts[4 * b].astype(np.float32)
                  + parts[4 * b + 1].astype(np.float32)
                  + parts[4 * b + 2].astype(np.float32)
                  + parts[4 * b + 3].astype(np.float32))
        out[b] += bo[None, :]
    return out


# revision 17
# speedup vs baseline: 1.0448x; 1.0200x over previous
"""Multi-head attention (shared QKV projection, floor-div scores) on 8 NeuronCores.

Problem: B=2, S=2048, HID=1024, NH=16, HD=64
    q = k = v = x @ Wq + bq          (reshaped to heads)
    scores = floor(q k^T / sqrt(64)) ; attn = softmax(scores)
    out = (attn v) @ Wo + bo

Sharding: core c handles batch c//4 and 4 heads ((c%4)*4 ..+4). Each core
computes its heads' contribution to out[b] = attn_out @ Wo; the host sums the
4 partials per batch and adds bo.

Device algorithm per core (fp16 matmuls with hi/lo split for full precision):
  - host pre-scales xT by 8^-0.5 (and bq by 8^-0.5, Wo by 8^0.5) so the PE
    score matmuls directly produce s/8 = scores/sqrt(HD); x and Wq are sent
    as fp16 hi + fp16 residual pairs.
  - qT pair tiles: 3-pass fp16 matmul (x16*w16 + dx*w16 + x16*dw) + bias-row
    matmul; PSUM (fp32) split into q16 (fp16) + dq (fp16 residual)
  - v tiles = PE-transposed q16 slices (+ ones column for rowsum Z)
  - per head pair, per q-quarter: scoresT blocks via 3 fp16 matmuls
    (q16*q16 + q16*dq + dq*q16), row-packed across the 2 heads (interleaved
    emission so the two heads' matmuls overlap in the PE array) ->
      floor via RNE trick on DVE: n = (s/8 + 63.5) + 2^23  ->
      P = exp(n - (2^23+64+PSHIFT)) fp16 on ACT (two i-blocks per op) ->
      oT[65, 512] += v_i^T @ P_i  (PE, ones col gives Z row)
    oT evicted to SBUF promptly (frees PSUM); rz = exp(-ln(Z)) (ACT),
    broadcast via DRAM round-trip DMA, oTn = oT * rz fp16 (GPSIMD)
  - partial = oTn_pair^T @ Wo_pair fp16 -> fp16 partial out
"""

import math
import sys

sys.path.insert(0, "/opt/trn_rl_repo")

import numpy as np
import concourse.bass as bass
import concourse.bacc as bacc
import concourse.tile as tile
from concourse import mybir
from concourse.bass_utils import run_bass_kernel_spmd

F32 = mybir.dt.float32
F16 = mybir.dt.float16
ADD = mybir.AluOpType.add
SUB = mybir.AluOpType.subtract
MULT = mybir.AluOpType.mult
AF = mybir.ActivationFunctionType

B, S, HID, NH, HD = 2, 2048, 1024, 16, 64
HPC = 4          # heads per core
NCORES = 8
KT = HID // 128  # 8 k-tiles
QT = S // 128    # 16 q/s tiles
C23 = float(2 ** 23)
PSHIFT = 10.0    # P = e^(n-PSHIFT); cancels in softmax; keeps P < fp16 max
SQ8 = 1.0 / math.sqrt(8.0)

_NC_CACHE = None


def _build():
    nc = bacc.Bacc("TRN2", target_bir_lowering=False, debug=False,
                   num_devices=NCORES)

    x16d = nc.dram_tensor("x16", [HID, S], F16, kind="ExternalInput")
    dx16d = nc.dram_tensor("dx16", [HID, S], F16, kind="ExternalInput")
    wqd, dwqd, wod, bqrd = [], [], [], []
    for p in range(2):
        wqd.append(nc.dram_tensor(f"wq{p}", [128, 1024], F16,
                                  kind="ExternalInput"))
        dwqd.append(nc.dram_tensor(f"dwq{p}", [128, 1024], F16,
                                   kind="ExternalInput"))
        wod.append(nc.dram_tensor(f"wo{p}", [128, 1024], F16,
                                  kind="ExternalInput"))
        bqrd.append(nc.dram_tensor(f"bqr{p}", [1, 128], F16,
                                   kind="ExternalInput"))
    ident = nc.dram_tensor("ident", [128, 64], F16, kind="ExternalInput")
    part = nc.dram_tensor("part", [S, HID], F16, kind="ExternalOutput")
    rzscr = nc.dram_tensor("rzscr", [HPC, S], F32)

    with tile.TileContext(nc) as tc:
        with (
            tc.tile_pool(name="cst", bufs=1) as cst,
            tc.tile_pool(name="big", bufs=1) as big,
            tc.tile_pool(name="wrk", bufs=3) as wrk,
            tc.tile_pool(name="osb", bufs=2) as osb,
            tc.tile_pool(name="zs", bufs=1) as zs,
            tc.tile_pool(name="ps_big", bufs=2, space="PSUM") as ps_big,
            tc.tile_pool(name="ps_small", bufs=3, space="PSUM") as ps_small,
        ):
            # ---- constants / inputs ----
            b_negc = cst.tile([128, 1], F32, tag="b_negc")
            nc.vector.memset(b_negc[:], -(C23 + 64.0 + PSHIFT))
            onesr = cst.tile([1, S], F16, tag="onesr")
            nc.vector.memset(onesr[:], 1.0)
            id_t = cst.tile([128, 64], F16, tag="id_t")
            nc.sync.dma_start(id_t[:], ident[:])
            wq_t, dwq_t, wo_t, bqr_t = [], [], [], []
            for p in range(2):
                w = cst.tile([128, 1024], F16, tag=f"wq_t{p}")
                nc.sync.dma_start(w[:], wqd[p][:])
                wq_t.append(w)
                w = cst.tile([128, 1024], F16, tag=f"dwq_t{p}")
                nc.sync.dma_start(w[:], dwqd[p][:])
                dwq_t.append(w)
                w = cst.tile([128, 1024], F16, tag=f"wo_t{p}")
                nc.sync.dma_start(w[:], wod[p][:])
                wo_t.append(w)
                w = cst.tile([1, 128], F16, tag=f"bqr_t{p}")
                nc.sync.dma_start(w[:], bqrd[p][:])
                bqr_t.append(w)
            x16_t, dx16_t = [], []
            for t in range(KT):
                w = big.tile([128, S], F16, tag=f"x16_{t}")
                nc.sync.dma_start(w[:], x16d[t * 128:(t + 1) * 128, :])
                x16_t.append(w)
            for t in range(KT):
                w = big.tile([128, S], F16, tag=f"dx16_{t}")
                nc.sync.dma_start(w[:], dx16d[t * 128:(t + 1) * 128, :])
                dx16_t.append(w)

            # ---- phase 1: qT projection (3-pass fp16) -> q16 + dq ----
            q16_t, dq_t = [], []
            for p in range(2):
                w = big.tile([128, S], F16, tag=f"q16_{p}")
                q16_t.append(w)
                w = big.tile([128, S], F16, tag=f"dq_{p}")
                dq_t.append(w)
            def emit_proj(p):
                for jh in range(2):
                    o = jh * 1024
                    pq = ps_small.tile([128, 1024], F32, tag="smallps",
                                       name=f"pq{p}_{jh}")
                    for pa, (lw, rx) in enumerate(
                            [(wq_t[p], x16_t), (dwq_t[p], x16_t),
                             (wq_t[p], dx16_t)]):
                        for t in range(KT):
                            for c in range(2):
                                nc.tensor.matmul(
                                    pq[:, c * 512:(c + 1) * 512],
                                    lw[:, t * 128:(t + 1) * 128],
                                    rx[t][:, o + c * 512:o + (c + 1) * 512],
                                    start=(pa == 0 and t == 0), stop=False,
                                )
                    for c in range(2):
                        nc.tensor.matmul(
                            pq[:, c * 512:(c + 1) * 512],
                            bqr_t[p][:],
                            onesr[:, o + c * 512:o + (c + 1) * 512],
                            start=False, stop=(c == 1),
                        )
                    nc.scalar.activation(q16_t[p][:, o:o + 1024], pq[:],
                                         AF.Identity, bias=0.0, scale=1.0)
                    nc.vector.tensor_tensor(dq_t[p][:, o:o + 1024], pq[:],
                                            q16_t[p][:, o:o + 1024], SUB)

            # ---- phase 2: v tiles (transposed q16 + ones col) ----
            v_t = [None] * HPC

            def emit_v(h):
                p, r = h // 2, (h % 2) * 64
                vt = big.tile([128, QT * 65], F16, tag=f"v{h}")
                for half in range(2):
                    pv = ps_small.tile([128, 512], F16, tag="smallps",
                                       name=f"pv{h}_{half}")
                    for tt in range(8):
                        i = half * 8 + tt
                        nc.tensor.transpose(
                            pv[:, tt * 64:(tt + 1) * 64],
                            q16_t[p][r:r + 64, i * 128:(i + 1) * 128],
                            id_t[r:r + 64, :],
                        )
                    dst = (vt[:, half * 520:half * 520 + 520]
                           .rearrange("p (t e) -> p t e", e=65)[:, :, 0:64])
                    src = pv[:, 0:512].rearrange("p (t e) -> p t e", e=64)
                    nc.vector.tensor_copy(dst, src)
                ones = vt[:].rearrange("p (t e) -> p t e", e=65)[:, :, 64:65]
                nc.vector.memset(ones, 1.0)
                v_t[h] = vt

            # ---- phase 3: per head-pair, per q-quarter ----
            oTn_t = []
            for p in range(2):
                w = big.tile([128, S], F16, tag=f"oTn{p}")
                oTn_t.append(w)

            for pp in range(2):
                emit_proj(pp)
                emit_v(2 * pp)
                emit_v(2 * pp + 1)
                q16 = q16_t[pp]
                dq = dq_t[pp]
                oTsb = []
                for hx in range(2):
                    w = osb.tile([65, S], F32, tag="oTsb",
                                 name=f"oTsb{pp}_{hx}")
                    oTsb.append(w)
                for J in range(4):
                    jo = J * 512
                    poT = []
                    for hx in range(2):
                        w = ps_big.tile([65, 512], F32, tag="bigps",
                                        name=f"poT{pp}_{J}_{hx}")
                        poT.append(w)
                    nt = None
                    for i in range(QT):
                        half = i % 2
                        if half == 0:
                            nt = wrk.tile([128, 2048], F32, tag="ntile")
                        pss = ps_small.tile([128, 1024], F32, tag="smallps",
                                            name=f"pss{pp}_{J}_{i}")
                        itile = slice(i * 128, (i + 1) * 128)
                        cs = slice(jo, jo + 512)
                        # 3-pass scores, interleaved across the head pair so
                        # the two row-groups overlap in the PE array
                        for pa, (wsel, rsel) in enumerate(
                                [(q16, q16), (q16, dq), (dq, q16)]):
                            for hx in range(2):
                                rs = slice(hx * 64, hx * 64 + 64)
                                nc.tensor.matmul(
                                    pss[:, hx * 512:(hx + 1) * 512],
                                    wsel[rs, itile], rsel[rs, cs],
                                    start=(pa == 0), stop=(pa == 2),
                                )
                        nc.vector.tensor_scalar(
                            nt[:, half * 1024:(half + 1) * 1024], pss[:],
                            63.5, C23, ADD, ADD,
                        )
                        if half == 1:
                            pt = wrk.tile([128, 2048], F16, tag="ptile")
                            nc.scalar.activation(pt[:], nt[:], AF.Exp,
                                                 bias=b_negc[:], scale=1.0)
                            for ii, hx in ((i - 1, 0), (i - 1, 1),
                                           (i, 0), (i, 1)):
                                h = 2 * pp + hx
                                hf = (ii % 2) * 1024
                                nc.tensor.matmul(
                                    poT[hx][:],
                                    v_t[h][:, ii * 65:(ii + 1) * 65],
                                    pt[:, hf + hx * 512:hf + (hx + 1) * 512],
                                    start=(ii == 0), stop=(ii == QT - 1),
                                )
                    for hx in range(2):
                        if (J + hx) % 2 == 0:
                            nc.scalar.copy(oTsb[hx][:, jo:jo + 512], poT[hx][:])
                        else:
                            nc.vector.tensor_copy(oTsb[hx][:, jo:jo + 512],
                                                  poT[hx][:])

                # normalization per head of this pair
                for hx in range(2):
                    h = 2 * pp + hx
                    r = hx * 64
                    lnz = zs.tile([1, S], F32, tag="lnz")
                    nc.scalar.activation(lnz[:], oTsb[hx][64:65, :], AF.Ln,
                                         bias=0.0, scale=1.0)
                    rz = zs.tile([1, S], F32, tag="rz")
                    nc.scalar.activation(rz[:], lnz[:], AF.Exp,
                                         bias=0.0, scale=-1.0)
                    if pp == 0:
                        # off the critical path: DMA round-trip broadcast
                        nc.sync.dma_start(rzscr[h:h + 1, :], rz[:])
                        repz = zs.tile([64, S], F32, tag="repz")
                        nc.sync.dma_start(
                            repz[:], rzscr[h:h + 1, :].broadcast_to([64, S]))
                        nc.gpsimd.tensor_tensor(oTn_t[pp][r:r + 64, :],
                                                oTsb[hx][0:64, :], repz[:],
                                                MULT)
                    else:
                        # tail-critical: PE rank-1 broadcast + DVE multiply
                        rz16 = zs.tile([1, S], F16, tag="rz16")
                        nc.vector.tensor_copy(rz16[:], rz[:])
                        for jh in range(2):
                            o2 = jh * 1024
                            prz = ps_small.tile([64, 1024], F32, tag="smallps",
                                                name=f"prz{hx}_{jh}")
                            for c in range(2):
                                nc.tensor.matmul(
                                    prz[:, c * 512:(c + 1) * 512],
                                    onesr[0:1, 0:64],
                                    rz16[:, o2 + c * 512:o2 + (c + 1) * 512],
                                    start=True, stop=True,
                                )
                            nc.vector.tensor_tensor(
                                oTn_t[pp][r:r + 64, o2:o2 + 1024],
                                prz[:], oTsb[hx][0:64, o2:o2 + 1024], MULT)

            # ---- phase 4: output projection (fp16) ----
            for m in range(QT):
                po = ps_small.tile([128, 1024], F32, tag="smallps",
                                   name=f"po{m}")
                for c in range(2):
                    nc.tensor.matmul(
                        po[:, c * 512:(c + 1) * 512],
                        oTn_t[0][:, m * 128:(m + 1) * 128],
                        wo_t[0][:, c * 512:(c + 1) * 512],
                        start=True, stop=False,
                    )
                    nc.tensor.matmul(
                        po[:, c * 512:(c + 1) * 512],
                        oTn_t[1][:, m * 128:(m + 1) * 128],
                        wo_t[1][:, c * 512:(c + 1) * 512],
                        start=False, stop=True,
                    )
                ot = wrk.tile([128, 1024], F16, tag="ptile", name=f"ostage{m}")
                if m % 2 == 0:
                    nc.vector.tensor_copy(ot[:], po[:])
                else:
                    nc.scalar.copy(ot[:], po[:])
                nc.sync.dma_start(part[m * 128:(m + 1) * 128, :], ot[:])

    nc.finalize()
    return nc


def _get_nc():
    global _NC_CACHE
    if _NC_CACHE is None:
        _NC_CACHE = _build()
    return _NC_CACHE


def make_in_maps(x, Wq, bq, Wo):
    eye = np.eye(64, dtype=np.float16)
    ident = np.vstack([eye, eye])
    in_maps = []
    for c in range(NCORES):
        b, hb = c // 4, (c % 4) * HPC
        xts = np.ascontiguousarray(x[b].T) * np.float32(SQ8)   # [1024, 2048]
        x16 = xts.astype(np.float16)
        dx16 = (xts - x16.astype(np.float32)).astype(np.float16)
        m = {"x16": x16, "dx16": dx16, "ident": ident}
        for p in range(2):
            lo = (hb + 2 * p) * HD          # first col/row of this head pair
            wq_cols = Wq[:, lo:lo + 128]    # [1024, 128]
            # lhsT k-tile layout: [128 part, 8 ktiles x 128]
            wqp = np.ascontiguousarray(
                wq_cols.reshape(KT, 128, 128).transpose(1, 0, 2).reshape(128, 1024)
            )
            w16 = wqp.astype(np.float16)
            m[f"wq{p}"] = w16
            m[f"dwq{p}"] = (wqp - w16.astype(np.float32)).astype(np.float16)
            m[f"wo{p}"] = (np.ascontiguousarray(Wo[lo:lo + 128, :])
                           * np.float32(1.0 / SQ8)).astype(np.float16)
            m[f"bqr{p}"] = (bq[None, lo:lo + 128]
                            * np.float32(SQ8)).astype(np.float16)
        in_maps.append(m)
    return in_maps


def kernel(x, Wq, bq, Wo, bo):
    x = np.asarray(x, np.float32)
    Wq = np.asarray(Wq, np.float32)
    bq = np.asarray(bq, np.float32)
    Wo = np.asarray(Wo, np.float32)
    bo = np.asarray(bo, np.float32)

    in_maps = make_in_maps(x, Wq, bq, Wo)
    res = run_bass_kernel_spmd(_get_nc(), in_maps, list(range(NCORES)))
    parts = [r["part"] for r in res.results]
    out = np.empty((B, S, HID), np.float32)
    for b in range(B):
        out[b] = (parts[4 * b].astype(np.float32)
                  + parts[4 * b + 1].astype(np.float32)
                  + parts[4 * b + 2].astype(np.float32)
                  + parts[4 * b + 3].astype(np.float32))
        out[b] += bo[None, :]
    return out
